# revision 11
# baseline (speedup 1.0000x reference)
"""Trainium2 kernel: X = inv(phi + sigma2*A) for the DeepKernelPacketGP module.

Math: B = phi + sigma2*A is exactly pentadiagonal, so X = inv(B) is
semiseparable: X[i,j] = F[i,:] @ C[:,j] for i < j (rank 2), G[i,:] @ D[:,j]
for i > j, plus the diagonal. Host (f64, O(n)): banded solves for F, G
(columns 0,1 and n-2,n-1 of X), batched local 5x5 solves per column for
C, D, diag. Device (8 cores, column-slab sharding): per 128x512 output
tile one K=4 fp32r matmul (per-row-block QR-orthonormalized generators),
PSUM->SBUF copy, DMA out. Diagonal 128x128 blocks are host-precomputed in
f64 and overwritten after the matmul; each core processes its 4 diagonal
tiles first so the overwrite offset is core-invariant (SPMD), and the host
unscrambles the row order on gather.
"""
import sys
sys.path.insert(0, '/opt/trn_rl_repo')
import numpy as np
from scipy.linalg import solve_banded

N = 4096
NCORES = 8
SLAB = N // NCORES         # 512
TB = 128                   # tile rows
NT = N // TB               # 32 tiles per slab

# ============================================================================
# Host math (float64)
# ============================================================================

def _stage1_bands(x, rho, sigma2):
    n = x.shape[0]; k = 5; m = 2; n_pow = 2
    c = np.sqrt(3.0) / rho
    W = n - 4
    idx = np.arange(W)[:, None] + np.arange(k)[None, :]
    xw = x[idx]
    t = xw - (xw[:, :1] + xw[:, -1:]) / 2
    pw = t[:, :, None] ** np.arange(n_pow)
    pos = pw * np.exp(c * t)[:, :, None]
    neg = pw * np.exp(-c * t)[:, :, None]
    e_first = np.zeros((W, 1, k)); e_first[:, :, 0] = 1.0
    Amat = np.concatenate([np.swapaxes(pos, 1, 2), np.swapaxes(neg, 1, 2), e_first], axis=1)
    rhs = np.zeros((k,)); rhs[-1] = 1.0
    a = np.linalg.solve(Amat, np.broadcast_to(rhs, (W, k))[..., None])[..., 0]
    d = np.abs(xw[:, :, None] - xw[:, None, :]); s = c * d
    Kw = (1 + s) * np.exp(-s)
    phiv = np.einsum('wij,wj->wi', Kw, a)
    bcol = phiv + sigma2 * a
    Bcols = np.zeros((n, 5))
    Bcols[2:n-2, :] = bcol
    def bnd(xseg, tshift, npos, nneg):
        ss = xseg.shape[0]
        xt = xseg + tshift
        rows = [xt**j * np.exp(c*xt) for j in range(npos)]
        rows += [xt**j * np.exp(-c*xt) for j in range(nneg)]
        e = np.zeros(ss); e[0] = 1.0
        rows.append(e)
        M = np.stack(rows); r = np.zeros(ss); r[-1] = 1.0
        aa = np.linalg.solve(M, r)
        dd = np.abs(xseg[:, None] - xseg[None, :]); s2 = c*dd
        return aa, ((1+s2)*np.exp(-s2)) @ aa
    for i in range(m):
        s_l = i + m + 1
        aa, pp = bnd(x[:s_l], -x[s_l-1], n_pow, s_l - 3)
        for r in range(s_l):
            Bcols[i, r - i + 2] = pp[r] + sigma2*aa[r]
        s_r = k - 1 - i
        aa, pp = bnd(x[n-s_r:], -x[n-s_r], s_r - 3, n_pow)
        col = n - m + i
        for ridx in range(s_r):
            r = n - s_r + ridx
            Bcols[col, r - col + 2] = pp[ridx] + sigma2*aa[ridx]
    return Bcols


def _host_generators(Bcols):
    """F, G (n x 2), C, D (2 x n), xd (n): semiseparable generators of inv(B)."""
    n = N
    ab = Bcols.T.copy()              # scipy banded form: ab[2+d, c] = B[c+d, c]
    E = np.zeros((n, 4))
    E[n-2, 0] = 1; E[n-1, 1] = 1; E[0, 2] = 1; E[1, 3] = 1
    sol = solve_banded((2, 2), ab, E)
    F = sol[:, 0:2]; G = sol[:, 2:4]

    Brow = np.zeros((5, n))          # Brow[2+d, j] = B[j, j+d]
    for d in range(-2, 3):
        jj = np.arange(max(0, -d), n - max(0, d))
        Brow[2 + d, jj] = Bcols[jj + d, 2 - d]

    def BofRI(r, i):
        d = i - r
        out = np.zeros(r.shape)
        ok = (np.abs(d) <= 2) & (r >= 0) & (r < n) & (i >= 0) & (i < n)
        out[ok] = Brow[2 + d[ok], r[ok]]
        return out

    # interior columns j=2..n-3: 5x5 solve for [c1, c2, xjj, d1, d2]
    jj = np.arange(2, n - 2)
    M = np.zeros((jj.size, 5, 5))
    for r_loc in range(5):
        r = jj - 2 + r_loc
        for t in range(-2, 3):
            i = r + t
            b = BofRI(r, i)
            ic = np.clip(i, 0, n - 1)
            lo = i < jj; eq = i == jj; hi = i > jj
            M[lo, r_loc, 0] += b[lo] * F[ic[lo], 0]
            M[lo, r_loc, 1] += b[lo] * F[ic[lo], 1]
            M[eq, r_loc, 2] += b[eq]
            M[hi, r_loc, 3] += b[hi] * G[ic[hi], 0]
            M[hi, r_loc, 4] += b[hi] * G[ic[hi], 1]
    rhs = np.zeros((jj.size, 5)); rhs[:, 2] = 1.0
    U = np.linalg.solve(M, rhs[..., None])[..., 0]
    C = np.zeros((2, n)); D = np.zeros((2, n)); xd = np.zeros(n)
    C[:, jj] = U[:, 0:2].T
    xd[jj] = U[:, 2]
    D[:, jj] = U[:, 3:5].T

    # boundary columns
    M3 = np.zeros((3, 3)); r3 = np.arange(3)
    for t in range(-2, 3):
        i = r3 + t; b = BofRI(r3, i); ic = np.clip(i, 0, n - 1)
        M3[:, 0] += b * (i == 0)
        for k in range(2):
            M3[:, 1 + k] += b * G[ic, k] * (i > 0)
    u = np.linalg.solve(M3, np.eye(3)[0])
    xd[0] = u[0]; D[:, 0] = u[1:3]

    M4 = np.zeros((4, 4)); r4 = np.arange(4)
    for t in range(-2, 3):
        i = r4 + t; b = BofRI(r4, i); ic = np.clip(i, 0, n - 1)
        M4[:, 0] += b * (i == 0)
        M4[:, 1] += b * (i == 1)
        for k in range(2):
            M4[:, 2 + k] += b * G[ic, k] * (i > 1)
    u = np.linalg.solve(M4, np.eye(4)[1])
    xd[1] = u[1]; D[:, 1] = u[2:4]
    # X[0,1] needed for the (0,0) diagonal center block
    x01 = u[0]

    M4 = np.zeros((4, 4)); r4 = np.arange(n - 4, n)
    for t in range(-2, 3):
        i = r4 + t; b = BofRI(r4, i); ic = np.clip(i, 0, n - 1)
        for k in range(2):
            M4[:, k] += b * F[ic, k] * (i < n - 2)
        M4[:, 2] += b * (i == n - 2)
        M4[:, 3] += b * (i == n - 1)
    u = np.linalg.solve(M4, np.eye(4)[2])
    C[:, n - 2] = u[0:2]; xd[n - 2] = u[2]
    xn12 = u[3]   # X[n-1, n-2]

    M3 = np.zeros((3, 3)); r3 = np.arange(n - 3, n)
    for t in range(-2, 3):
        i = r3 + t; b = BofRI(r3, i); ic = np.clip(i, 0, n - 1)
        for k in range(2):
            M3[:, k] += b * F[ic, k] * (i < n - 1)
        M3[:, 2] += b * (i == n - 1)
    u = np.linalg.solve(M3, np.eye(3)[2])
    C[:, n - 1] = u[0:2]; xd[n - 1] = u[2]

    return F, G, C, D, xd, x01, xn12


def _host_pieces(Bcols):
    F, G, C, D, xd, x01, xn12 = _host_generators(Bcols)
    n = N
    # per-row-block QR bases
    Qf = np.zeros((NT, TB, 2)); Rf = np.zeros((NT, 2, 2))
    Qg = np.zeros((NT, TB, 2)); Rg = np.zeros((NT, 2, 2))
    for g in range(NT):
        rows = slice(TB * g, TB * (g + 1))
        Qf[g], Rf[g] = np.linalg.qr(F[rows])
        Qg[g], Rg[g] = np.linalg.qr(G[rows])
    # diagonal center blocks (f64)
    cen = np.zeros((NT, TB, TB))
    ii = np.arange(TB)[:, None]; jc = np.arange(TB)[None, :]
    for g in range(NT):
        rows = slice(TB * g, TB * (g + 1))
        up = F[rows] @ C[:, rows]
        loP = G[rows] @ D[:, rows]
        blk = np.where(ii < jc, up, np.where(ii > jc, loP, 0.0))
        blk[np.arange(TB), np.arange(TB)] = xd[TB * g:TB * (g + 1)]
        cen[g] = blk
    # exact corner values not covered by the rank-2 reps
    cen[0][0, 1] = x01
    cen[NT - 1][TB - 1, TB - 2] = xn12
    return dict(F=F, G=G, C=C, D=D, xd=xd, Qf=Qf, Rf=Rf, Qg=Qg, Rg=Rg, cen=cen)


def _tile_order(core):
    """Processing order of row-blocks for this core: its 4 diag blocks first."""
    own = [4 * core + k for k in range(4)]
    rest = [g for g in range(NT) if g // 4 != core]
    return own + rest


def _core_inputs(P, core):
    from ml_dtypes import bfloat16
    cols = slice(SLAB * core, SLAB * (core + 1))
    order = _tile_order(core)
    lhsT = np.zeros((NT, 4, TB), np.float64)
    rhs = np.zeros((NT, 4, SLAB), np.float64)
    colblk = (np.arange(SLAB * core, SLAB * (core + 1)) // TB)  # global 128-block
    for t, g in enumerate(order):
        lhsT[t, 0:2] = P["Qf"][g].T
        lhsT[t, 2:4] = P["Qg"][g].T
        mC = (colblk > g).astype(np.float64)
        mD = (colblk < g).astype(np.float64)
        rhs[t, 0:2] = (P["Rf"][g] @ P["C"][:, cols]) * mC
        rhs[t, 2:4] = (P["Rg"][g] @ P["D"][:, cols]) * mD
    diag = np.zeros((TB, 4 * TB), np.float64)
    for k in range(4):
        diag[:, k*TB:(k+1)*TB] = P["cen"][4 * core + k]
    return {"lhsT": lhsT.astype(bfloat16), "rhs": rhs.astype(bfloat16),
            "diag": diag.astype(bfloat16)}


# ============================================================================
# Device kernel
# ============================================================================

_CACHED = {}

def _build_nc():
    import concourse.bass as bass
    import concourse.mybir as mybir
    import concourse.tile as tile
    from concourse.vector_clock import ScopedClock

    def _patched_drain_and_barrier(self, tick_clock, wait_clock):
        nopw = self.nc.gpsimd.nop()
        wait_clock.add_sem_waits(nopw.ins, ScopedClock({None: tick_clock.global_clock}))
        waits = list(nopw.ins.sync_info.on_wait) if nopw.ins.sync_info else []
        if len(waits) > 1:
            nopw.ins.sync_info.on_wait = waits[:1]
            for w in waits[1:]:
                extra = self.nc.gpsimd.nop()
                extra.ins.sync_info = mybir.SyncInfo(on_wait=[w], on_update=[])
        self.nc.sync.drain()
        self.nc.all_engine_barrier()
        assert self.sems is not None
        popped = self.nc._tile_sem_poison_stack.pop()
        assert popped is self._sem_poison
        self.nc.clear_and_free_semaphores(list(self.sems.allocated().values()))
        self.nc.all_engine_barrier()
    tile.TileContext._drain_and_barrier = _patched_drain_and_barrier

    F32 = mybir.dt.float32
    BF16 = mybir.dt.bfloat16
    S = SLAB
    AP = bass.AP

    nc = bass.Bass(target_bir_lowering=False)
    dins = {
        "lhsT": nc.dram_tensor("lhsT", [NT, 4, TB], BF16, kind="ExternalInput"),
        "rhs": nc.dram_tensor("rhs", [NT, 4, S], BF16, kind="ExternalInput"),
        "diag": nc.dram_tensor("diag", [TB, 4 * TB], BF16, kind="ExternalInput"),
    }
    dout = nc.dram_tensor("xslab", [N, S], BF16, kind="ExternalOutput")

    with tile.TileContext(nc) as tc:
        with tc.tile_pool(name="main", bufs=1) as pool, \
             tc.tile_pool(name="io", bufs=2) as iopool, \
             tc.tile_pool(name="ps", bufs=8, space="PSUM") as pspool:
            # inputs: t-major DRAM layout so each partition's data is many
            # short runs -> packets spread across all DMA engines
            lhs = pool.tile([4, NT * TB], BF16, tag="lhs")
            src = dins["lhsT"][:]
            nc.sync.dma_start(
                AP(lhs[:].tensor, lhs[:].offset,
                   [[NT * TB, 4], [TB, NT], [1, TB]]),
                AP(src.tensor, src.offset, [[TB, 4], [4 * TB, NT], [1, TB]]))
            rhsb = pool.tile([4, NT * S], BF16, tag="rhsb")
            src = dins["rhs"][:]
            nc.sync.dma_start(
                AP(rhsb[:].tensor, rhsb[:].offset,
                   [[NT * S, 4], [S, NT], [1, S]]),
                AP(src.tensor, src.offset, [[S, 4], [4 * S, NT], [1, S]]))
            dg = pool.tile([TB, 4 * TB], BF16, tag="dg")
            nc.sync.dma_start(dg[:], dins["diag"][:])
            for grp in range(NT // 4):
                ob = iopool.tile([TB, 4 * S], BF16, tag="ob")
                for k in range(4):
                    t = 4 * grp + k
                    ps = pspool.tile([TB, S], F32, tag="ps")
                    nc.tensor.matmul(
                        ps[:],
                        lhs[:, t*TB:(t+1)*TB],
                        rhsb[:, t*S:(t+1)*S],
                    )
                    dst = ob[:, k*S:(k+1)*S]
                    if k % 2 == 0:
                        nc.scalar.copy(dst, ps[:])
                    else:
                        nc.vector.tensor_copy(dst, ps[:])
                if grp == 0:
                    # overwrite the 4 diagonal 128x128 blocks in one strided copy
                    ob_ap = ob[:]
                    dg_ap = dg[:]
                    nc.vector.tensor_copy(
                        AP(ob_ap.tensor, ob_ap.offset,
                           [[4 * S, TB], [S + TB, 4], [1, TB]]),
                        AP(dg_ap.tensor, dg_ap.offset,
                           [[4 * TB, TB], [TB, 4], [1, TB]]))
                # one DMA for 4 tiles: dram rows [512*grp, 512*(grp+1))
                ob_ap = ob[:]
                dout_ap = dout[:]
                nc.sync.dma_start(
                    AP(dout_ap.tensor, 4 * grp * TB * S,
                       [[S, TB], [TB * S, 4], [1, S]]),
                    AP(ob_ap.tensor, ob_ap.offset,
                       [[4 * S, TB], [S, 4], [1, S]]))

    # --- post-pass: this walrus build allows only 1 sync-wait per
    # instruction; split extras onto preceding same-engine NOPs ---
    def _split_waits(maxw=1):
        all_bbs = list(nc.main_func.blocks)
        for bb in all_bbs:
            out = []
            for inst in bb.instructions:
                si = getattr(inst, "sync_info", None)
                ow = list(si.on_wait) if (si is not None and si.on_wait) else []
                if len(ow) > maxw:
                    si.on_wait = ow[-maxw:]
                    try:
                        eng_builder = nc.engines[inst.engine]
                    except Exception:
                        eng_builder = nc.sync
                    for w in ow[:-maxw]:
                        nop = eng_builder.nop()
                        for bb2 in nc.main_func.blocks:
                            li = bb2.instructions
                            if li and li[-1] is nop.ins:
                                li.pop()
                                break
                        nop.ins.sync_info = mybir.SyncInfo(on_wait=[w], on_update=[])
                        out.append(nop.ins)
                out.append(inst)
            bb.instructions[:] = out
    _split_waits()
    return nc, dins, dout


def _device_run(P):
    from concourse.bass_utils import run_bass_kernel_spmd
    if "nc" not in _CACHED:
        _CACHED["nc"] = _build_nc()
    nc, dins, dout = _CACHED["nc"]
    in_maps = [_core_inputs(P, core) for core in range(NCORES)]
    res = run_bass_kernel_spmd(nc, in_maps, list(range(NCORES)))
    X = np.zeros((N, N), np.float32)
    for core in range(NCORES):
        slab = res.results[core]["xslab"]
        order = _tile_order(core)
        for t, g in enumerate(order):
            X[TB*g:TB*(g+1), SLAB*core:SLAB*(core+1)] = slab[TB*t:TB*(t+1)]
    return X


def kernel(x, rho, sigma2):
    x = np.asarray(x, dtype=np.float64)
    rho = float(np.asarray(rho)); sigma2 = float(np.asarray(sigma2))
    Bcols = _stage1_bands(x, rho, sigma2)
    P = _host_pieces(Bcols)
    _CACHED["P_obj"] = P
    X = _device_run(P).astype(np.float64)
    return X


# revision 17
# speedup vs baseline: 1.1011x; 1.1011x over previous
"""Trainium2 kernel: X = inv(phi + sigma2*A) for the DeepKernelPacketGP module.

Math: B = phi + sigma2*A is exactly pentadiagonal, so X = inv(B) is
semiseparable: X[i,j] = F[i,:] @ C[:,j] for i < j (rank 2), G[i,:] @ D[:,j]
for i > j, plus the diagonal. Host (f64, O(n)): banded solves for F, G
(columns 0,1 and n-2,n-1 of X), batched local 5x5 solves per column for
C, D, diag. Device (8 cores, column-slab sharding): per 128x512 output
tile one K=4 fp32r matmul (per-row-block QR-orthonormalized generators),
PSUM->SBUF copy, DMA out. Diagonal 128x128 blocks are host-precomputed in
f64 and overwritten after the matmul; each core processes its 4 diagonal
tiles first so the overwrite offset is core-invariant (SPMD), and the host
unscrambles the row order on gather.
"""
import sys
sys.path.insert(0, '/opt/trn_rl_repo')
import numpy as np
from scipy.linalg import solve_banded

N = 4096
NCORES = 8
SLAB = N // NCORES         # 512
TB = 128                   # tile rows
NT = N // TB               # 32 tiles per slab

# ============================================================================
# Host math (float64)
# ============================================================================

def _stage1_bands(x, rho, sigma2):
    n = x.shape[0]; k = 5; m = 2; n_pow = 2
    c = np.sqrt(3.0) / rho
    W = n - 4
    idx = np.arange(W)[:, None] + np.arange(k)[None, :]
    xw = x[idx]
    t = xw - (xw[:, :1] + xw[:, -1:]) / 2
    pw = t[:, :, None] ** np.arange(n_pow)
    pos = pw * np.exp(c * t)[:, :, None]
    neg = pw * np.exp(-c * t)[:, :, None]
    e_first = np.zeros((W, 1, k)); e_first[:, :, 0] = 1.0
    Amat = np.concatenate([np.swapaxes(pos, 1, 2), np.swapaxes(neg, 1, 2), e_first], axis=1)
    rhs = np.zeros((k,)); rhs[-1] = 1.0
    a = np.linalg.solve(Amat, np.broadcast_to(rhs, (W, k))[..., None])[..., 0]
    d = np.abs(xw[:, :, None] - xw[:, None, :]); s = c * d
    Kw = (1 + s) * np.exp(-s)
    phiv = np.einsum('wij,wj->wi', Kw, a)
    bcol = phiv + sigma2 * a
    Bcols = np.zeros((n, 5))
    Bcols[2:n-2, :] = bcol
    def bnd(xseg, tshift, npos, nneg):
        ss = xseg.shape[0]
        xt = xseg + tshift
        rows = [xt**j * np.exp(c*xt) for j in range(npos)]
        rows += [xt**j * np.exp(-c*xt) for j in range(nneg)]
        e = np.zeros(ss); e[0] = 1.0
        rows.append(e)
        M = np.stack(rows); r = np.zeros(ss); r[-1] = 1.0
        aa = np.linalg.solve(M, r)
        dd = np.abs(xseg[:, None] - xseg[None, :]); s2 = c*dd
        return aa, ((1+s2)*np.exp(-s2)) @ aa
    for i in range(m):
        s_l = i + m + 1
        aa, pp = bnd(x[:s_l], -x[s_l-1], n_pow, s_l - 3)
        for r in range(s_l):
            Bcols[i, r - i + 2] = pp[r] + sigma2*aa[r]
        s_r = k - 1 - i
        aa, pp = bnd(x[n-s_r:], -x[n-s_r], s_r - 3, n_pow)
        col = n - m + i
        for ridx in range(s_r):
            r = n - s_r + ridx
            Bcols[col, r - col + 2] = pp[ridx] + sigma2*aa[ridx]
    return Bcols


def _host_generators(Bcols):
    """F, G (n x 2), C, D (2 x n), xd (n): semiseparable generators of inv(B)."""
    n = N
    ab = Bcols.T.copy()              # scipy banded form: ab[2+d, c] = B[c+d, c]
    E = np.zeros((n, 4))
    E[n-2, 0] = 1; E[n-1, 1] = 1; E[0, 2] = 1; E[1, 3] = 1
    sol = solve_banded((2, 2), ab, E)
    F = sol[:, 0:2]; G = sol[:, 2:4]

    Brow = np.zeros((5, n))          # Brow[2+d, j] = B[j, j+d]
    for d in range(-2, 3):
        jj = np.arange(max(0, -d), n - max(0, d))
        Brow[2 + d, jj] = Bcols[jj + d, 2 - d]

    def BofRI(r, i):
        d = i - r
        out = np.zeros(r.shape)
        ok = (np.abs(d) <= 2) & (r >= 0) & (r < n) & (i >= 0) & (i < n)
        out[ok] = Brow[2 + d[ok], r[ok]]
        return out

    # interior columns j=2..n-3: 5x5 solve for [c1, c2, xjj, d1, d2]
    jj = np.arange(2, n - 2)
    M = np.zeros((jj.size, 5, 5))
    for r_loc in range(5):
        r = jj - 2 + r_loc
        for t in range(-2, 3):
            i = r + t
            b = BofRI(r, i)
            ic = np.clip(i, 0, n - 1)
            lo = i < jj; eq = i == jj; hi = i > jj
            M[lo, r_loc, 0] += b[lo] * F[ic[lo], 0]
            M[lo, r_loc, 1] += b[lo] * F[ic[lo], 1]
            M[eq, r_loc, 2] += b[eq]
            M[hi, r_loc, 3] += b[hi] * G[ic[hi], 0]
            M[hi, r_loc, 4] += b[hi] * G[ic[hi], 1]
    rhs = np.zeros((jj.size, 5)); rhs[:, 2] = 1.0
    U = np.linalg.solve(M, rhs[..., None])[..., 0]
    C = np.zeros((2, n)); D = np.zeros((2, n)); xd = np.zeros(n)
    C[:, jj] = U[:, 0:2].T
    xd[jj] = U[:, 2]
    D[:, jj] = U[:, 3:5].T

    # boundary columns
    M3 = np.zeros((3, 3)); r3 = np.arange(3)
    for t in range(-2, 3):
        i = r3 + t; b = BofRI(r3, i); ic = np.clip(i, 0, n - 1)
        M3[:, 0] += b * (i == 0)
        for k in range(2):
            M3[:, 1 + k] += b * G[ic, k] * (i > 0)
    u = np.linalg.solve(M3, np.eye(3)[0])
    xd[0] = u[0]; D[:, 0] = u[1:3]

    M4 = np.zeros((4, 4)); r4 = np.arange(4)
    for t in range(-2, 3):
        i = r4 + t; b = BofRI(r4, i); ic = np.clip(i, 0, n - 1)
        M4[:, 0] += b * (i == 0)
        M4[:, 1] += b * (i == 1)
        for k in range(2):
            M4[:, 2 + k] += b * G[ic, k] * (i > 1)
    u = np.linalg.solve(M4, np.eye(4)[1])
    xd[1] = u[1]; D[:, 1] = u[2:4]
    # X[0,1] needed for the (0,0) diagonal center block
    x01 = u[0]

    M4 = np.zeros((4, 4)); r4 = np.arange(n - 4, n)
    for t in range(-2, 3):
        i = r4 + t; b = BofRI(r4, i); ic = np.clip(i, 0, n - 1)
        for k in range(2):
            M4[:, k] += b * F[ic, k] * (i < n - 2)
        M4[:, 2] += b * (i == n - 2)
        M4[:, 3] += b * (i == n - 1)
    u = np.linalg.solve(M4, np.eye(4)[2])
    C[:, n - 2] = u[0:2]; xd[n - 2] = u[2]
    xn12 = u[3]   # X[n-1, n-2]

    M3 = np.zeros((3, 3)); r3 = np.arange(n - 3, n)
    for t in range(-2, 3):
        i = r3 + t; b = BofRI(r3, i); ic = np.clip(i, 0, n - 1)
        for k in range(2):
            M3[:, k] += b * F[ic, k] * (i < n - 1)
        M3[:, 2] += b * (i == n - 1)
    u = np.linalg.solve(M3, np.eye(3)[2])
    C[:, n - 1] = u[0:2]; xd[n - 1] = u[2]

    return F, G, C, D, xd, x01, xn12


def _host_pieces(Bcols):
    F, G, C, D, xd, x01, xn12 = _host_generators(Bcols)
    n = N
    # per-row-block QR bases
    Qf = np.zeros((NT, TB, 2)); Rf = np.zeros((NT, 2, 2))
    Qg = np.zeros((NT, TB, 2)); Rg = np.zeros((NT, 2, 2))
    for g in range(NT):
        rows = slice(TB * g, TB * (g + 1))
        Qf[g], Rf[g] = np.linalg.qr(F[rows])
        Qg[g], Rg[g] = np.linalg.qr(G[rows])
    # diagonal center blocks (f64)
    cen = np.zeros((NT, TB, TB))
    ii = np.arange(TB)[:, None]; jc = np.arange(TB)[None, :]
    for g in range(NT):
        rows = slice(TB * g, TB * (g + 1))
        up = F[rows] @ C[:, rows]
        loP = G[rows] @ D[:, rows]
        blk = np.where(ii < jc, up, np.where(ii > jc, loP, 0.0))
        blk[np.arange(TB), np.arange(TB)] = xd[TB * g:TB * (g + 1)]
        cen[g] = blk
    # exact corner values not covered by the rank-2 reps
    cen[0][0, 1] = x01
    cen[NT - 1][TB - 1, TB - 2] = xn12
    return dict(F=F, G=G, C=C, D=D, xd=xd, Qf=Qf, Rf=Rf, Qg=Qg, Rg=Rg, cen=cen)


def _tile_order(core):
    """Processing order of row-blocks for this core: its 4 diag blocks first."""
    own = [4 * core + k for k in range(4)]
    rest = [g for g in range(NT) if g // 4 != core]
    return own + rest


def _core_inputs(P, core):
    from ml_dtypes import bfloat16
    cols = slice(SLAB * core, SLAB * (core + 1))
    order = _tile_order(core)
    # PE row-tiling layout: tile t (slot i=t%4, wave w=t//4) stores its
    # K=4 rows on partitions 32*i + 4*w + k; lhsT is a full zero-padded
    # [128, 8*TB] image (zero weight rows kill garbage rhs partitions).
    lhsT = np.zeros((128, 8 * TB), np.float64)
    rhs = np.zeros((128, SLAB), np.float64)
    colblk = (np.arange(SLAB * core, SLAB * (core + 1)) // TB)  # global 128-block
    for t, g in enumerate(order):
        i, w = t % 4, t // 4
        lk = np.concatenate([P["Qf"][g].T, P["Qg"][g].T], axis=0)  # [4, TB]
        mC = (colblk > g).astype(np.float64)
        mD = (colblk < g).astype(np.float64)
        rk = np.concatenate([(P["Rf"][g] @ P["C"][:, cols]) * mC,
                             (P["Rg"][g] @ P["D"][:, cols]) * mD], axis=0)
        p0 = 32 * i + 4 * w
        lhsT[p0:p0 + 4, w * TB:(w + 1) * TB] = lk
        rhs[p0:p0 + 4, :] = rk
    diag = np.zeros((TB, 4 * TB), np.float64)
    for k in range(4):
        diag[:, k*TB:(k+1)*TB] = P["cen"][4 * core + k]
    return {"lhsT": lhsT.astype(bfloat16), "rhs": rhs.astype(bfloat16),
            "diag": diag.astype(bfloat16)}


# ============================================================================
# Device kernel
# ============================================================================

_CACHED = {}

def _build_nc():
    import concourse.bass as bass
    import concourse.mybir as mybir
    import concourse.tile as tile
    from concourse.vector_clock import ScopedClock

    def _patched_drain_and_barrier(self, tick_clock, wait_clock):
        nopw = self.nc.gpsimd.nop()
        wait_clock.add_sem_waits(nopw.ins, ScopedClock({None: tick_clock.global_clock}))
        waits = list(nopw.ins.sync_info.on_wait) if nopw.ins.sync_info else []
        if len(waits) > 1:
            nopw.ins.sync_info.on_wait = waits[:1]
            for w in waits[1:]:
                extra = self.nc.gpsimd.nop()
                extra.ins.sync_info = mybir.SyncInfo(on_wait=[w], on_update=[])
        self.nc.sync.drain()
        self.nc.all_engine_barrier(sem_only=True)
        assert self.sems is not None
        popped = self.nc._tile_sem_poison_stack.pop()
        assert popped is self._sem_poison
        self.nc.clear_and_free_semaphores(list(self.sems.allocated().values()))
        self.nc.all_engine_barrier(sem_only=True)
    tile.TileContext._drain_and_barrier = _patched_drain_and_barrier

    F32 = mybir.dt.float32
    BF16 = mybir.dt.bfloat16
    S = SLAB
    AP = bass.AP

    nc = bass.Bass(target_bir_lowering=False)
    dins = {
        "lhsT": nc.dram_tensor("lhsT", [128, 8 * TB], BF16, kind="ExternalInput"),
        "rhs": nc.dram_tensor("rhs", [128, S], BF16, kind="ExternalInput"),
        "diag": nc.dram_tensor("diag", [TB, 4 * TB], BF16, kind="ExternalInput"),
    }
    dout = nc.dram_tensor("xslab", [N, S], BF16, kind="ExternalOutput")

    with tile.TileContext(nc) as tc:
        with tc.tile_pool(name="main", bufs=1) as pool, \
             tc.tile_pool(name="io", bufs=2) as iopool, \
             tc.tile_pool(name="ps", bufs=8, space="PSUM") as pspool:
            # PE row-tiling layout: tile t (slot i=t%4, wave w=t//4) has its
            # K=4 rows on partitions 32i+4w+k -> inputs cover all 128
            # partitions (all 16 DMA engines); zero lhsT rows kill the
            # garbage rhs partitions inside each K=32 operand window.
            lhs = pool.tile([128, 8 * TB], BF16, tag="lhs")
            nc.sync.dma_start(lhs[:], dins["lhsT"][:])
            rhsb = pool.tile([128, S], BF16, tag="rhsb")
            nc.sync.dma_start(rhsb[:], dins["rhs"][:])
            dg = pool.tile([TB, 4 * TB], BF16, tag="dg")
            nc.sync.dma_start(dg[:], dins["diag"][:])
            for grp in range(NT // 4):
                ob = iopool.tile([TB, 4 * S], BF16, tag="ob")
                w = grp
                for k in range(4):
                    i = k
                    ps = pspool.tile([TB, S], F32, tag="ps")
                    nc.tensor.matmul(
                        ps[:],
                        lhs[32*i:32*i+32, w*TB:(w+1)*TB],
                        rhsb[32*i:32*i+32, :],
                        tile_position=(32 * i, 0),
                    )
                    dst = ob[:, k*S:(k+1)*S]
                    if k == 0:
                        nc.scalar.copy(dst, ps[:])
                    else:
                        nc.vector.tensor_copy(dst, ps[:])
                if grp == 0:
                    # overwrite the 4 diagonal 128x128 blocks in one strided copy
                    ob_ap = ob[:]
                    dg_ap = dg[:]
                    nc.scalar.copy(
                        AP(ob_ap.tensor, ob_ap.offset,
                           [[4 * S, TB], [S + TB, 4], [1, TB]]),
                        AP(dg_ap.tensor, dg_ap.offset,
                           [[4 * TB, TB], [TB, 4], [1, TB]]))
                # one DMA for 4 tiles: dram rows [512*grp, 512*(grp+1))
                ob_ap = ob[:]
                dout_ap = dout[:]
                nc.sync.dma_start(
                    AP(dout_ap.tensor, 4 * grp * TB * S,
                       [[S, TB], [TB * S, 4], [1, S]]),
                    AP(ob_ap.tensor, ob_ap.offset,
                       [[4 * S, TB], [S, 4], [1, S]]))

    # --- post-pass: this walrus build allows only 1 sync-wait per
    # instruction; split extras onto preceding same-engine NOPs ---
    def _split_waits(maxw=1):
        all_bbs = list(nc.main_func.blocks)
        for bb in all_bbs:
            out = []
            for inst in bb.instructions:
                si = getattr(inst, "sync_info", None)
                ow = list(si.on_wait) if (si is not None and si.on_wait) else []
                if len(ow) > maxw:
                    si.on_wait = ow[-maxw:]
                    try:
                        eng_builder = nc.engines[inst.engine]
                    except Exception:
                        eng_builder = nc.sync
                    for w in ow[:-maxw]:
                        nop = eng_builder.nop()
                        for bb2 in nc.main_func.blocks:
                            li = bb2.instructions
                            if li and li[-1] is nop.ins:
                                li.pop()
                                break
                        nop.ins.sync_info = mybir.SyncInfo(on_wait=[w], on_update=[])
                        out.append(nop.ins)
                out.append(inst)
            bb.instructions[:] = out
    _split_waits()
    return nc, dins, dout


def _device_run(P):
    from concourse.bass_utils import run_bass_kernel_spmd
    if "nc" not in _CACHED:
        _CACHED["nc"] = _build_nc()
    nc, dins, dout = _CACHED["nc"]
    in_maps = [_core_inputs(P, core) for core in range(NCORES)]
    res = run_bass_kernel_spmd(nc, in_maps, list(range(NCORES)))
    X = np.zeros((N, N), np.float32)
    for core in range(NCORES):
        slab = res.results[core]["xslab"]
        order = _tile_order(core)
        for t, g in enumerate(order):
            X[TB*g:TB*(g+1), SLAB*core:SLAB*(core+1)] = slab[TB*t:TB*(t+1)]
    return X


def kernel(x, rho, sigma2):
    x = np.asarray(x, dtype=np.float64)
    rho = float(np.asarray(rho)); sigma2 = float(np.asarray(sigma2))
    Bcols = _stage1_bands(x, rho, sigma2)
    P = _host_pieces(Bcols)
    _CACHED["P_obj"] = P
    X = _device_run(P).astype(np.float64)
    return X


# revision 19
# speedup vs baseline: 1.1536x; 1.0477x over previous
"""Trainium2 kernel: X = inv(phi + sigma2*A) for the DeepKernelPacketGP module.

Math: B = phi + sigma2*A is exactly pentadiagonal, so X = inv(B) is
semiseparable: X[i,j] = F[i,:] @ C[:,j] for i < j (rank 2), G[i,:] @ D[:,j]
for i > j, plus the diagonal. Host (f64, O(n)): banded solves for F, G
(columns 0,1 and n-2,n-1 of X), batched local 5x5 solves per column for
C, D, diag. Device (8 cores, column-slab sharding): per 128x512 output
tile one K=4 fp32r matmul (per-row-block QR-orthonormalized generators),
PSUM->SBUF copy, DMA out. Diagonal 128x128 blocks are host-precomputed in
f64 and overwritten after the matmul; each core processes its 4 diagonal
tiles first so the overwrite offset is core-invariant (SPMD), and the host
unscrambles the row order on gather.
"""
import sys
sys.path.insert(0, '/opt/trn_rl_repo')
import numpy as np
from scipy.linalg import solve_banded

N = 4096
NCORES = 8
SLAB = N // NCORES         # 512
TB = 128                   # tile rows
NT = N // TB               # 32 tiles per slab

# ============================================================================
# Host math (float64)
# ============================================================================

def _stage1_bands(x, rho, sigma2):
    n = x.shape[0]; k = 5; m = 2; n_pow = 2
    c = np.sqrt(3.0) / rho
    W = n - 4
    idx = np.arange(W)[:, None] + np.arange(k)[None, :]
    xw = x[idx]
    t = xw - (xw[:, :1] + xw[:, -1:]) / 2
    pw = t[:, :, None] ** np.arange(n_pow)
    pos = pw * np.exp(c * t)[:, :, None]
    neg = pw * np.exp(-c * t)[:, :, None]
    e_first = np.zeros((W, 1, k)); e_first[:, :, 0] = 1.0
    Amat = np.concatenate([np.swapaxes(pos, 1, 2), np.swapaxes(neg, 1, 2), e_first], axis=1)
    rhs = np.zeros((k,)); rhs[-1] = 1.0
    a = np.linalg.solve(Amat, np.broadcast_to(rhs, (W, k))[..., None])[..., 0]
    d = np.abs(xw[:, :, None] - xw[:, None, :]); s = c * d
    Kw = (1 + s) * np.exp(-s)
    phiv = np.einsum('wij,wj->wi', Kw, a)
    bcol = phiv + sigma2 * a
    Bcols = np.zeros((n, 5))
    Bcols[2:n-2, :] = bcol
    def bnd(xseg, tshift, npos, nneg):
        ss = xseg.shape[0]
        xt = xseg + tshift
        rows = [xt**j * np.exp(c*xt) for j in range(npos)]
        rows += [xt**j * np.exp(-c*xt) for j in range(nneg)]
        e = np.zeros(ss); e[0] = 1.0
        rows.append(e)
        M = np.stack(rows); r = np.zeros(ss); r[-1] = 1.0
        aa = np.linalg.solve(M, r)
        dd = np.abs(xseg[:, None] - xseg[None, :]); s2 = c*dd
        return aa, ((1+s2)*np.exp(-s2)) @ aa
    for i in range(m):
        s_l = i + m + 1
        aa, pp = bnd(x[:s_l], -x[s_l-1], n_pow, s_l - 3)
        for r in range(s_l):
            Bcols[i, r - i + 2] = pp[r] + sigma2*aa[r]
        s_r = k - 1 - i
        aa, pp = bnd(x[n-s_r:], -x[n-s_r], s_r - 3, n_pow)
        col = n - m + i
        for ridx in range(s_r):
            r = n - s_r + ridx
            Bcols[col, r - col + 2] = pp[ridx] + sigma2*aa[ridx]
    return Bcols


def _host_generators(Bcols):
    """F, G (n x 2), C, D (2 x n), xd (n): semiseparable generators of inv(B)."""
    n = N
    ab = Bcols.T.copy()              # scipy banded form: ab[2+d, c] = B[c+d, c]
    E = np.zeros((n, 4))
    E[n-2, 0] = 1; E[n-1, 1] = 1; E[0, 2] = 1; E[1, 3] = 1
    sol = solve_banded((2, 2), ab, E)
    F = sol[:, 0:2]; G = sol[:, 2:4]

    Brow = np.zeros((5, n))          # Brow[2+d, j] = B[j, j+d]
    for d in range(-2, 3):
        jj = np.arange(max(0, -d), n - max(0, d))
        Brow[2 + d, jj] = Bcols[jj + d, 2 - d]

    def BofRI(r, i):
        d = i - r
        out = np.zeros(r.shape)
        ok = (np.abs(d) <= 2) & (r >= 0) & (r < n) & (i >= 0) & (i < n)
        out[ok] = Brow[2 + d[ok], r[ok]]
        return out

    # interior columns j=2..n-3: 5x5 solve for [c1, c2, xjj, d1, d2]
    jj = np.arange(2, n - 2)
    M = np.zeros((jj.size, 5, 5))
    for r_loc in range(5):
        r = jj - 2 + r_loc
        for t in range(-2, 3):
            i = r + t
            b = BofRI(r, i)
            ic = np.clip(i, 0, n - 1)
            lo = i < jj; eq = i == jj; hi = i > jj
            M[lo, r_loc, 0] += b[lo] * F[ic[lo], 0]
            M[lo, r_loc, 1] += b[lo] * F[ic[lo], 1]
            M[eq, r_loc, 2] += b[eq]
            M[hi, r_loc, 3] += b[hi] * G[ic[hi], 0]
            M[hi, r_loc, 4] += b[hi] * G[ic[hi], 1]
    rhs = np.zeros((jj.size, 5)); rhs[:, 2] = 1.0
    U = np.linalg.solve(M, rhs[..., None])[..., 0]
    C = np.zeros((2, n)); D = np.zeros((2, n)); xd = np.zeros(n)
    C[:, jj] = U[:, 0:2].T
    xd[jj] = U[:, 2]
    D[:, jj] = U[:, 3:5].T

    # boundary columns
    M3 = np.zeros((3, 3)); r3 = np.arange(3)
    for t in range(-2, 3):
        i = r3 + t; b = BofRI(r3, i); ic = np.clip(i, 0, n - 1)
        M3[:, 0] += b * (i == 0)
        for k in range(2):
            M3[:, 1 + k] += b * G[ic, k] * (i > 0)
    u = np.linalg.solve(M3, np.eye(3)[0])
    xd[0] = u[0]; D[:, 0] = u[1:3]

    M4 = np.zeros((4, 4)); r4 = np.arange(4)
    for t in range(-2, 3):
        i = r4 + t; b = BofRI(r4, i); ic = np.clip(i, 0, n - 1)
        M4[:, 0] += b * (i == 0)
        M4[:, 1] += b * (i == 1)
        for k in range(2):
            M4[:, 2 + k] += b * G[ic, k] * (i > 1)
    u = np.linalg.solve(M4, np.eye(4)[1])
    xd[1] = u[1]; D[:, 1] = u[2:4]
    # X[0,1] needed for the (0,0) diagonal center block
    x01 = u[0]

    M4 = np.zeros((4, 4)); r4 = np.arange(n - 4, n)
    for t in range(-2, 3):
        i = r4 + t; b = BofRI(r4, i); ic = np.clip(i, 0, n - 1)
        for k in range(2):
            M4[:, k] += b * F[ic, k] * (i < n - 2)
        M4[:, 2] += b * (i == n - 2)
        M4[:, 3] += b * (i == n - 1)
    u = np.linalg.solve(M4, np.eye(4)[2])
    C[:, n - 2] = u[0:2]; xd[n - 2] = u[2]
    xn12 = u[3]   # X[n-1, n-2]

    M3 = np.zeros((3, 3)); r3 = np.arange(n - 3, n)
    for t in range(-2, 3):
        i = r3 + t; b = BofRI(r3, i); ic = np.clip(i, 0, n - 1)
        for k in range(2):
            M3[:, k] += b * F[ic, k] * (i < n - 1)
        M3[:, 2] += b * (i == n - 1)
    u = np.linalg.solve(M3, np.eye(3)[2])
    C[:, n - 1] = u[0:2]; xd[n - 1] = u[2]

    return F, G, C, D, xd, x01, xn12


def _host_pieces(Bcols):
    F, G, C, D, xd, x01, xn12 = _host_generators(Bcols)
    n = N
    # per-row-block QR bases
    Qf = np.zeros((NT, TB, 2)); Rf = np.zeros((NT, 2, 2))
    Qg = np.zeros((NT, TB, 2)); Rg = np.zeros((NT, 2, 2))
    for g in range(NT):
        rows = slice(TB * g, TB * (g + 1))
        Qf[g], Rf[g] = np.linalg.qr(F[rows])
        Qg[g], Rg[g] = np.linalg.qr(G[rows])
    # diagonal center blocks (f64)
    cen = np.zeros((NT, TB, TB))
    ii = np.arange(TB)[:, None]; jc = np.arange(TB)[None, :]
    for g in range(NT):
        rows = slice(TB * g, TB * (g + 1))
        up = F[rows] @ C[:, rows]
        loP = G[rows] @ D[:, rows]
        blk = np.where(ii < jc, up, np.where(ii > jc, loP, 0.0))
        blk[np.arange(TB), np.arange(TB)] = xd[TB * g:TB * (g + 1)]
        cen[g] = blk
    # exact corner values not covered by the rank-2 reps
    cen[0][0, 1] = x01
    cen[NT - 1][TB - 1, TB - 2] = xn12
    return dict(F=F, G=G, C=C, D=D, xd=xd, Qf=Qf, Rf=Rf, Qg=Qg, Rg=Rg, cen=cen)


def _tile_order(core):
    """Processing order of row-blocks for this core: its 4 diag blocks first."""
    own = [4 * core + k for k in range(4)]
    rest = [g for g in range(NT) if g // 4 != core]
    return own + rest


def _core_inputs(P, core):
    from ml_dtypes import bfloat16
    cols = slice(SLAB * core, SLAB * (core + 1))
    order = _tile_order(core)
    # PE row-tiling layout: tile t (slot i=t%4, wave w=t//4) stores its
    # K=4 rows on partitions 32*i + 4*w + k; lhsT is a full zero-padded
    # [128, 8*TB] image (zero weight rows kill garbage rhs partitions).
    lhsT = np.zeros((128, 8 * TB), np.float64)
    rhs = np.zeros((128, SLAB), np.float64)
    colblk = (np.arange(SLAB * core, SLAB * (core + 1)) // TB)  # global 128-block
    for t, g in enumerate(order):
        i, w = t % 4, t // 4
        lk = np.concatenate([P["Qf"][g].T, P["Qg"][g].T], axis=0)  # [4, TB]
        mC = (colblk > g).astype(np.float64)
        mD = (colblk < g).astype(np.float64)
        rk = np.concatenate([(P["Rf"][g] @ P["C"][:, cols]) * mC,
                             (P["Rg"][g] @ P["D"][:, cols]) * mD], axis=0)
        p0 = 32 * i + 4 * w
        lhsT[p0:p0 + 4, w * TB:(w + 1) * TB] = lk
        rhs[p0:p0 + 4, :] = rk
    diag = np.zeros((TB, 4 * TB), np.float64)
    for k in range(4):
        diag[:, k*TB:(k+1)*TB] = P["cen"][4 * core + k]
    return {"lhsT": lhsT.astype(bfloat16), "rhs": rhs.astype(bfloat16),
            "diag": diag.astype(bfloat16)}


# ============================================================================
# Device kernel
# ============================================================================

_CACHED = {}

def _build_nc():
    import concourse.bass as bass
    import concourse.mybir as mybir
    import concourse.tile as tile
    from concourse.vector_clock import ScopedClock

    def _patched_drain_and_barrier(self, tick_clock, wait_clock):
        nopw = self.nc.gpsimd.nop()
        wait_clock.add_sem_waits(nopw.ins, ScopedClock({None: tick_clock.global_clock}))
        waits = list(nopw.ins.sync_info.on_wait) if nopw.ins.sync_info else []
        if len(waits) > 1:
            nopw.ins.sync_info.on_wait = waits[:1]
            for w in waits[1:]:
                extra = self.nc.gpsimd.nop()
                extra.ins.sync_info = mybir.SyncInfo(on_wait=[w], on_update=[])
        self.nc.sync.drain()
        self.nc.all_engine_barrier(sem_only=True)
        assert self.sems is not None
        popped = self.nc._tile_sem_poison_stack.pop()
        assert popped is self._sem_poison
        self.nc.clear_and_free_semaphores(list(self.sems.allocated().values()))
        self.nc.all_engine_barrier(sem_only=True)
    tile.TileContext._drain_and_barrier = _patched_drain_and_barrier

    F32 = mybir.dt.float32
    BF16 = mybir.dt.bfloat16
    S = SLAB
    AP = bass.AP

    nc = bass.Bass(target_bir_lowering=False)
    dins = {
        "lhsT": nc.dram_tensor("lhsT", [128, 8 * TB], BF16, kind="ExternalInput"),
        "rhs": nc.dram_tensor("rhs", [128, S], BF16, kind="ExternalInput"),
        "diag": nc.dram_tensor("diag", [TB, 4 * TB], BF16, kind="ExternalInput"),
    }
    dout = nc.dram_tensor("xslab", [N, S], BF16, kind="ExternalOutput")

    with tile.TileContext(nc) as tc:
        with tc.tile_pool(name="main", bufs=1) as pool, \
             tc.tile_pool(name="io", bufs=2) as iopool, \
             tc.tile_pool(name="ps", bufs=2, space="PSUM") as pspool:
            # PE row-tiling layout: tile t (slot i=t%4, wave w=t//4) has its
            # K=4 rows on partitions 32i+4w+k -> inputs cover all 128
            # partitions (all 16 DMA engines); zero lhsT rows kill the
            # garbage rhs partitions inside each K=32 operand window.
            lhs = pool.tile([128, 8 * TB], BF16, tag="lhs")
            nc.sync.dma_start(lhs[:], dins["lhsT"][:])
            rhsb = pool.tile([128, S], BF16, tag="rhsb")
            nc.sync.dma_start(rhsb[:], dins["rhs"][:])
            dg = pool.tile([TB, 4 * TB], BF16, tag="dg")
            nc.sync.dma_start(dg[:], dins["diag"][:])
            for w in range(8):
                if w % 2 == 0:
                    ob = iopool.tile([TB, 8 * S], BF16, tag="ob")
                half = ob[:, (w % 2) * 4 * S:((w % 2) + 1) * 4 * S]
                ps = pspool.tile([TB, 4 * S], F32, tag="ps")
                for i in range(4):
                    nc.tensor.matmul(
                        ps[:, i*S:(i+1)*S],
                        lhs[32*i:32*i+32, w*TB:(w+1)*TB],
                        rhsb[32*i:32*i+32, :],
                        tile_position=(32 * i, 0),
                    )
                if w % 2 == 0:
                    nc.scalar.copy(half, ps[:])
                else:
                    nc.vector.tensor_copy(half, ps[:])
                if w == 0:
                    # overwrite the 4 diagonal 128x128 blocks in one strided copy
                    ob_ap = ob[:]
                    dg_ap = dg[:]
                    nc.scalar.copy(
                        AP(ob_ap.tensor, ob_ap.offset,
                           [[8 * S, TB], [S + TB, 4], [1, TB]]),
                        AP(dg_ap.tensor, dg_ap.offset,
                           [[4 * TB, TB], [TB, 4], [1, TB]]))
                if w % 2 == 1:
                    # one DMA for 8 tiles: dram rows [1024*(w//2), ...)
                    ob_ap = ob[:]
                    dout_ap = dout[:]
                    nc.sync.dma_start(
                        AP(dout_ap.tensor, (w - 1) * 4 * TB * S,
                           [[S, TB], [TB * S, 8], [1, S]]),
                        AP(ob_ap.tensor, ob_ap.offset,
                           [[8 * S, TB], [S, 8], [1, S]]))

    # --- post-pass: this walrus build allows only 1 sync-wait per
    # instruction; split extras onto preceding same-engine NOPs ---
    def _split_waits(maxw=1):
        all_bbs = list(nc.main_func.blocks)
        for bb in all_bbs:
            out = []
            for inst in bb.instructions:
                si = getattr(inst, "sync_info", None)
                ow = list(si.on_wait) if (si is not None and si.on_wait) else []
                if len(ow) > maxw:
                    si.on_wait = ow[-maxw:]
                    try:
                        eng_builder = nc.engines[inst.engine]
                    except Exception:
                        eng_builder = nc.sync
                    for w in ow[:-maxw]:
                        nop = eng_builder.nop()
                        for bb2 in nc.main_func.blocks:
                            li = bb2.instructions
                            if li and li[-1] is nop.ins:
                                li.pop()
                                break
                        nop.ins.sync_info = mybir.SyncInfo(on_wait=[w], on_update=[])
                        out.append(nop.ins)
                out.append(inst)
            bb.instructions[:] = out
    _split_waits()
    return nc, dins, dout


def _device_run(P):
    from concourse.bass_utils import run_bass_kernel_spmd
    if "nc" not in _CACHED:
        _CACHED["nc"] = _build_nc()
    nc, dins, dout = _CACHED["nc"]
    in_maps = [_core_inputs(P, core) for core in range(NCORES)]
    res = run_bass_kernel_spmd(nc, in_maps, list(range(NCORES)))
    X = np.zeros((N, N), np.float32)
    for core in range(NCORES):
        slab = res.results[core]["xslab"]
        order = _tile_order(core)
        for t, g in enumerate(order):
            X[TB*g:TB*(g+1), SLAB*core:SLAB*(core+1)] = slab[TB*t:TB*(t+1)]
    return X


def kernel(x, rho, sigma2):
    x = np.asarray(x, dtype=np.float64)
    rho = float(np.asarray(rho)); sigma2 = float(np.asarray(sigma2))
    Bcols = _stage1_bands(x, rho, sigma2)
    P = _host_pieces(Bcols)
    _CACHED["P_obj"] = P
    X = _device_run(P).astype(np.float64)
    return X
